# Initial kernel scaffold
#
"""Trainium2 Bass kernel for nn_ChessGraphPooling (segment_reduce).

Data-parallel over whole graphs: 4096 boards x 64 nodes sharded across 8
NeuronCores (512 graphs / 32768 nodes per core); small weights replicated.
Per core, nodes are processed in "supertiles" of 512 nodes (4 partition
chunks of 128; 8 graphs), grouped in "megatiles" of 8 supertiles so the
segment softmax and LN-rstd work batch across 80 score rows at once.

Layouts:
  - x is kept in node-partition layout [128 nodes, C] (pooling matmuls and
    the strategic branch) and transposed on the PE into T-layout [C, nodes]
    tiles that feed the per-node linears as the moving operand.
  - scorer hidden activations are computed directly in T-layout so the
    scorer second layers become tiny matmuls and the segment softmax is a
    grouped free-axis reduction.
  - per-graph pooled features are staged in T-layout [C, graph*pool] so the
    post-pooling MLP stack runs as plain K-chunked matmuls.

Matmuls run as float32r (fp32 bits, full-rate rows at N>=256 on TRN2).
"""

import os
import sys

sys.path.insert(0, "/opt/trn_rl_repo")

from contextlib import ExitStack

import numpy as np

import concourse.bass as bass
import concourse.bacc as bacc
import concourse.tile as tile
import concourse.mybir as mybir
from concourse.bass_utils import run_bass_kernel_spmd
from concourse.masks import make_identity

F32 = mybir.dt.float32
# matmul dtype: float32r (full-rate rows) by default; K_MM_FP32=1 for plain fp32
F32R = mybir.dt.float32 if os.environ.get("K_MM_FP32") else mybir.dt.float32r
BF16 = mybir.dt.bfloat16
I32 = mybir.dt.int32
AF = mybir.ActivationFunctionType
OP = mybir.AluOpType
AX = mybir.AxisListType

C = 256
H = 8
NODES = 64
NEG = 0.2
N_CORES = 8
ST = 512          # nodes per supertile
CHUNKS = 4        # 128-node chunks per supertile
MEGA = 8          # supertiles per megatile (80 score rows)
FULL_N_MEGA = 8   # megatiles per core at full size


def _r(ap):
    return ap.bitcast(F32R)


def build_nc(n_mega=FULL_N_MEGA):
    nodes_pc = n_mega * MEGA * ST
    graphs_pc = nodes_pc // NODES
    assert graphs_pc % 128 == 0, "post stage needs graphs_pc multiple of 128"

    nc = bacc.Bacc("TRN2", num_devices=N_CORES)

    dt = {}

    def din(name, shape, dtype=F32):
        dt[name] = nc.dram_tensor(name, shape, dtype, kind="ExternalInput")

    din("x", [nodes_pc, C])
    din("nt", [nodes_pc], I32)
    din("w1", [2, 128, 512], F32R)
    din("b1a", [128, 4])
    din("pew", [2, 128, 256], F32R)
    din("b1p", [128, 2])
    din("w2c", [128, 4, 32], F32R)
    din("pw2c", [128, 2, 32], F32R)
    din("b2", [1, 32], F32R)
    din("spw", [2, 128, 256], F32R)
    din("spb", [1, 256], F32R)
    din("spg", [1, 256])
    din("spbt", [1, 256])
    din("sw", [64, 1])
    din("cw", [16, 128, 256], F32R)
    din("cb", [1, 256], F32R)
    din("cg", [1, 256])
    din("cbt", [1, 256])
    din("hw", [4, 128, 256], F32R)
    din("hb", [1, 256], F32R)
    din("hg", [1, 256])
    din("hbt", [1, 256])
    din("p1w", [6, 128, 512], F32R)
    din("p1b", [1, 512], F32R)
    din("p1g", [1, 512])
    din("p1bt", [1, 512])
    din("p2w", [4, 128, 256], F32R)
    din("p2b", [1, 256], F32R)
    out_d = nc.dram_tensor("out", [graphs_pc, C], F32, kind="ExternalOutput")

    with tile.TileContext(nc) as tc:
        _build_body(nc, tc, n_mega, graphs_pc, dt, out_d)
    nc.compile()
    return nc


def _bcast(nc, dst, src_d):
    """DMA a [1, D] dram row broadcast across dst's partitions."""
    nc.gpsimd.dma_start(
        out=dst, in_=src_d.ap().partition_broadcast(dst.shape[0])
    )


def _build_body(nc, tc, n_mega, graphs_pc, dt, out_d):
    gchunks = graphs_pc // 128

    with ExitStack() as top:
        consts = top.enter_context(tc.tile_pool(name="consts", bufs=1))
        persist = top.enter_context(tc.tile_pool(name="persist", bufs=1))

        # ---- constants for the main loop ----
        w1 = [consts.tile([128, 512], F32R, tag=f"w1_{k}", name=f"w1_{k}") for k in range(2)]
        pew = [consts.tile([128, 256], F32R, tag=f"pew_{k}", name=f"pew_{k}") for k in range(2)]
        spw = [consts.tile([128, 256], F32R, tag=f"spw_{k}", name=f"spw_{k}") for k in range(2)]
        for k in range(2):
            nc.sync.dma_start(out=w1[k], in_=dt["w1"].ap()[k])
            nc.sync.dma_start(out=pew[k], in_=dt["pew"].ap()[k])
            nc.sync.dma_start(out=spw[k], in_=dt["spw"].ap()[k])
        b1a = consts.tile([128, 4], F32)
        nc.sync.dma_start(out=b1a, in_=dt["b1a"].ap())
        b1p = consts.tile([128, 2], F32)
        nc.sync.dma_start(out=b1p, in_=dt["b1p"].ap())
        w2c = consts.tile([128, 4, 32], F32R)
        nc.sync.dma_start(out=w2c, in_=dt["w2c"].ap())
        pw2c = consts.tile([128, 2, 32], F32R)
        nc.sync.dma_start(out=pw2c, in_=dt["pw2c"].ap())
        b2 = consts.tile([1, 32], F32R)
        nc.sync.dma_start(out=b2, in_=dt["b2"].ap())
        spbR = consts.tile([1, 256], F32R)
        nc.sync.dma_start(out=spbR, in_=dt["spb"].ap())
        gB = consts.tile([128, 256], F32, tag="gB")
        _bcast(nc, gB, dt["spg"])
        btB = consts.tile([128, 256], F32, tag="btB")
        _bcast(nc, btB, dt["spbt"])

        sa = consts.tile([128, 1], F32, tag="sa")
        nc.sync.dma_start(out=sa[0:64, :], in_=dt["sw"].ap())
        nc.sync.dma_start(out=sa[64:128, :], in_=dt["sw"].ap())
        sa2 = consts.tile([128, 1], F32, tag="sa2")
        nc.vector.tensor_tensor(out=sa2, in0=sa, in1=sa, op=OP.mult)

        onesf = consts.tile([1, 512], F32, tag="onesf")
        nc.vector.memset(onesf, 1.0)
        ones512 = consts.tile([1, 512], F32R, tag="ones512")
        nc.vector.tensor_copy(out=ones512, in_=onesf)
        ones1 = consts.tile([1, 128], F32R, tag="ones1")
        nc.vector.tensor_copy(out=ones1, in_=onesf[:, 0:128])
        poolf = consts.tile([128, 32], F32, tag="poolf")
        nc.gpsimd.memset(poolf, 0.0)
        nc.gpsimd.memset(poolf[0:64, 0:1], 1.0 / NODES)
        nc.gpsimd.memset(poolf[64:128, 1:2], 1.0 / NODES)
        poolones = consts.tile([128, 32], F32R, tag="poolones")
        nc.vector.tensor_copy(out=poolones, in_=poolf)
        zerof = consts.tile([128, 32], F32, tag="zerof")
        nc.gpsimd.memset(zerof, 0.0)
        ident = consts.tile([128, 128], F32, tag="ident")
        make_identity(nc, ident)
        maskS = consts.tile([128, 512], F32, tag="maskS")
        nc.vector.memset(maskS, 1.0)

        # ---- persistent staging (alive into the post stage) ----
        staged_x = persist.tile([128, 2, graphs_pc * 10], F32R, tag="staged_x")
        staged_sf = persist.tile([128, 2, graphs_pc], F32R, tag="staged_sf")

        # ---- main loop ----
        with ExitStack() as main:
            xpool = main.enter_context(tc.tile_pool(name="xpool", bufs=9))
            xrp = main.enter_context(tc.tile_pool(name="xrp", bufs=2))
            t1pool = main.enter_context(tc.tile_pool(name="t1pool", bufs=9))
            actp = main.enter_context(tc.tile_pool(name="actp", bufs=4))
            sfp = main.enter_context(tc.tile_pool(name="sfp", bufs=3))
            scrp = main.enter_context(tc.tile_pool(name="scrp", bufs=3))
            wcp = main.enter_context(tc.tile_pool(name="wcp", bufs=3))
            megap = main.enter_context(tc.tile_pool(name="megap", bufs=2))
            pps = main.enter_context(tc.tile_pool(name="pps", bufs=2))

            ps_tp = main.enter_context(
                tc.tile_pool(name="ps_tp", bufs=2, space="PSUM"))
            ps_mm = main.enter_context(
                tc.tile_pool(name="ps_mm", bufs=2, space="PSUM"))
            ps_zs = main.enter_context(
                tc.tile_pool(name="ps_zs", bufs=1, space="PSUM"))
            ps_sc = main.enter_context(
                tc.tile_pool(name="ps_sc", bufs=1, space="PSUM"))
            ps_xs = main.enter_context(
                tc.tile_pool(name="ps_xs", bufs=2, space="PSUM"))

            for mega in range(n_mega):
                _mega_body(
                    nc, tc, mega, dt, staged_x, staged_sf,
                    w1, pew, spw, b1a, b1p, w2c, pw2c, b2, spbR, gB, btB,
                    sa, sa2, ones512, ones1, poolones, zerof, ident, maskS,
                    xpool, xrp, t1pool, actp, sfp, scrp, wcp, megap, pps,
                    ps_tp, ps_mm, ps_zs, ps_sc, ps_xs,
                )

        # ---- post stage ----
        with ExitStack() as post:
            posw = post.enter_context(tc.tile_pool(name="posw", bufs=1))
            pos = post.enter_context(tc.tile_pool(name="pos", bufs=4))
            posT = post.enter_context(tc.tile_pool(name="posT", bufs=1))
            ps_po = post.enter_context(
                tc.tile_pool(name="ps_po", bufs=2, space="PSUM"))
            ps_pz = post.enter_context(
                tc.tile_pool(name="ps_pz", bufs=2, space="PSUM"))
            ps_pt = post.enter_context(
                tc.tile_pool(name="ps_pt", bufs=2, space="PSUM"))
            _post_body(
                nc, tc, graphs_pc, gchunks, dt, staged_x, staged_sf,
                ones1, ident, posw, pos, posT, ps_po, ps_pz, ps_pt, out_d,
            )


def _mega_body(
    nc, tc, mega, dt, staged_x, staged_sf,
    w1, pew, spw, b1a, b1p, w2c, pw2c, b2, spbR, gB, btB,
    sa, sa2, ones512, ones1, poolones, zerof, ident, maskS,
    xpool, xrp, t1pool, actp, sfp, scrp, wcp, megap, pps,
    ps_tp, ps_mm, ps_zs, ps_sc, ps_xs,
):
    # masks: node_types -> maskS rows 10*j+8 (m) / 10*j+9 (1-m)
    ntm = megap.tile([8, 512], I32, tag="ntm")
    nc.sync.dma_start(
        out=ntm,
        in_=dt["nt"].ap()[mega * MEGA * ST:(mega + 1) * MEGA * ST]
        .rearrange("(s n) -> s n", s=8),
    )
    m8 = megap.tile([8, 512], F32, tag="m8")
    nc.vector.tensor_copy(out=m8, in_=ntm)
    n8 = megap.tile([8, 512], F32, tag="n8")
    nc.vector.tensor_scalar(
        out=n8, in0=m8, scalar1=-1.0, scalar2=1.0, op0=OP.mult, op1=OP.add
    )
    nc.sync.dma_start(out=maskS[8:88:10, :], in_=m8)
    nc.sync.dma_start(out=maskS[9:89:10, :], in_=n8)

    scstack = megap.tile([80, 512], F32, tag="scstack")
    mvs = megap.tile([128, MEGA, 4, 2], F32, tag="mvs")
    xs = []
    t1s = []
    for s8 in range(MEGA):
        s = mega * MEGA + s8
        xsb = xpool.tile([128, 4, 256], F32, tag="xsb")
        nc.sync.dma_start(
            out=xsb,
            in_=dt["x"].ap()[s * ST:(s + 1) * ST, :]
            .rearrange("(c p) m -> p c m", p=128),
        )
        xs.append(xsb)

        # transpose x chunks into T-layout [C, nodes]
        xT = [actp.tile([128, 512], F32R, tag="xT0", name="xT0"),
              actp.tile([128, 512], F32R, tag="xT1", name="xT1")]
        for c in range(CHUNKS):
            for k in range(2):
                tp = ps_tp.tile([128, 128], F32, tag="tp")
                nc.tensor.transpose(
                    tp, xsb[:, c, k * 128:(k + 1) * 128], ident
                )
                if (c * 2 + k) % 2 == 0:
                    nc.vector.tensor_copy(
                        out=xT[k][:, c * 128:(c + 1) * 128], in_=tp
                    )
                else:
                    nc.scalar.copy(
                        out=xT[k][:, c * 128:(c + 1) * 128], in_=tp
                    )

        scp = ps_sc.tile([32, 512], F32, tag="scp")
        # attention scorer: hT [hd, nodes] in 4 M-chunks; scores via matmul
        for m in range(4):
            ph = ps_mm.tile([128, 512], F32, tag="ph")
            nc.tensor.matmul(
                ph, _r(w1[0][:, m * 128:(m + 1) * 128]), _r(xT[0]),
                start=True, stop=False,
            )
            nc.tensor.matmul(
                ph, _r(w1[1][:, m * 128:(m + 1) * 128]), _r(xT[1]),
                start=False, stop=True,
            )
            hL = actp.tile([128, 512], F32R, tag="hL")
            nc.scalar.activation(
                out=hL, in_=ph, func=AF.Prelu,
                bias=b1a[:, m:m + 1], scale=1.0, alpha=NEG,
            )
            nc.tensor.matmul(
                scp, _r(w2c[:, m, :]), _r(hL),
                start=(m == 0), stop=False,
            )

        # piece/empty scorer: peT [pe-dim, nodes] in 2 M-chunks
        for m in range(2):
            pp = ps_mm.tile([128, 512], F32, tag="ph")
            nc.tensor.matmul(
                pp, _r(pew[0][:, m * 128:(m + 1) * 128]), _r(xT[0]),
                start=True, stop=False,
            )
            nc.tensor.matmul(
                pp, _r(pew[1][:, m * 128:(m + 1) * 128]), _r(xT[1]),
                start=False, stop=True,
            )
            peL = actp.tile([128, 512], F32R, tag="hL")
            nc.scalar.activation(
                out=peL, in_=pp, func=AF.Prelu,
                bias=b1p[:, m:m + 1], scale=1.0, alpha=NEG,
            )
            nc.tensor.matmul(
                scp, _r(pw2c[:, m, :]), _r(peL),
                start=False, stop=False,
            )
        nc.tensor.matmul(scp, _r(b2), _r(ones512), start=False, stop=True)
        stmp = scrp.tile([10, 512], F32, tag="stmp")
        nc.vector.tensor_copy(out=stmp, in_=scp[0:10, :])
        nc.sync.dma_start(out=scstack[s8 * 10:(s8 + 1) * 10, :], in_=stmp)

        # strategic branch: zs = x @ spW + spb (node layout), LN stats + center
        t1 = t1pool.tile([128, 4, 256], F32, tag="t1")
        t1s.append(t1)
        for cp in range(2):
            pz = ps_zs.tile([128, 512], F32, tag="pz")
            for half in range(2):
                c = cp * 2 + half
                sl = pz[:, half * 256:(half + 1) * 256]
                nc.tensor.matmul(
                    sl, _r(xT[0][:, c * 128:(c + 1) * 128]), _r(spw[0]),
                    start=True, stop=False,
                )
                nc.tensor.matmul(
                    sl, _r(xT[1][:, c * 128:(c + 1) * 128]), _r(spw[1]),
                    start=False, stop=False,
                )
                nc.tensor.matmul(
                    sl, _r(ones1), _r(spbR), start=False, stop=True,
                )
            for half in range(2):
                c = cp * 2 + half
                sl = pz[:, half * 256:(half + 1) * 256]
                st6 = scrp.tile([128, 6], F32, tag="st6")
                nc.vector.bn_stats(out=st6, in_=sl)
                nc.vector.bn_aggr(out=mvs[:, s8, c, :], in_=st6)
                # t1 = (zs - mu) * gamma  (sa-scaling folded into rho below)
                nc.vector.scalar_tensor_tensor(
                    out=t1[:, c], in0=sl, scalar=mvs[:, s8, c, 0:1],
                    in1=gB, op0=OP.subtract, op1=OP.mult,
                )

    # pe-score rows: (s + b2) * mask; att rows * 1.0
    nc.vector.tensor_tensor(
        out=scstack, in0=scstack, in1=maskS[0:80, :], op=OP.mult
    )

    # batched segment softmax over [80 rows, 8 graphs, 64 nodes]
    wT = megap.tile([80, 512], F32, tag="wT")
    mx = megap.tile([80, 8], F32, tag="mx")
    sc3 = scstack.rearrange("p (g n) -> p g n", n=NODES)
    wT3 = wT.rearrange("p (g n) -> p g n", n=NODES)
    nc.vector.tensor_reduce(out=mx, in_=sc3, axis=AX.X, op=OP.max)
    nc.vector.tensor_tensor(
        out=wT3, in0=sc3,
        in1=mx.unsqueeze(2).broadcast_to([80, 8, NODES]),
        op=OP.subtract,
    )
    nc.scalar.activation(out=wT, in_=wT, func=AF.Exp)
    dsum = megap.tile([80, 8], F32, tag="dsum")
    nc.vector.tensor_reduce(out=dsum, in_=wT3, axis=AX.X, op=OP.add)
    nc.vector.tensor_scalar(
        out=dsum, in0=dsum, scalar1=1e-16, scalar2=None, op0=OP.add,
    )
    nc.vector.reciprocal(out=dsum, in_=dsum)
    nc.vector.tensor_tensor(
        out=wT3, in0=wT3,
        in1=dsum.unsqueeze(2).broadcast_to([80, 8, NODES]),
        op=OP.mult,
    )

    # transpose the whole weight stack: wtt[:, c, r] = wT[r, c*128+:]
    wtt = megap.tile([128, 4, 80], F32, tag="wtt")
    for c in range(CHUNKS):
        tp = ps_tp.tile([128, 128], F32, tag="tp", name="wttp")
        nc.tensor.transpose(
            tp[:, 0:80], wT[:, c * 128:(c + 1) * 128], ident[0:80, 0:80]
        )
        nc.scalar.copy(out=wtt[:, c, :], in_=tp[:, 0:80])

    # batched LN scale: rho = sa / sqrt(sa^2 * var + eps)
    rho = megap.tile([128, 32], F32, tag="rho")
    nc.vector.tensor_scalar(
        out=rho, in0=mvs[:, :, :, 1].rearrange("p a b -> p (a b)"),
        scalar1=sa2, scalar2=1e-5, op0=OP.mult, op1=OP.add,
    )
    nc.scalar.activation(out=rho, in_=rho, func=AF.Sqrt)
    nc.vector.reciprocal(out=rho, in_=rho)
    nc.vector.tensor_scalar(
        out=rho, in0=rho, scalar1=sa, scalar2=None, op0=OP.mult
    )

    # phase C: finish strat LN, pool x and sf per supertile
    for s8 in range(MEGA):
        s = mega * MEGA + s8
        xsb = xs[s8]
        t1 = t1s[s8]
        xr = xrp.tile([128, 4, 256], F32R, tag="xr")
        nc.gpsimd.tensor_copy(out=xr, in_=xsb)
        for c in range(CHUNKS):
            # sf = relu(t1 * rho + beta)
            t2 = scrp.tile([128, 256], F32, tag="t2")
            nc.vector.scalar_tensor_tensor(
                out=t2, in0=t1[:, c],
                scalar=rho[:, s8 * 4 + c:s8 * 4 + c + 1],
                in1=btB, op0=OP.mult, op1=OP.add,
            )
            sf = sfp.tile([128, 256], F32R, tag="sf")
            nc.scalar.activation(out=sf, in_=t2, func=AF.Relu)
            pc = ps_xs.tile([32, 512], F32, tag="pc")
            nc.tensor.matmul(
                pc[0:32, 256:512], _r(poolones), _r(sf),
                start=True, stop=True,
            )
            # pooling weight columns for this chunk (block-diag over graphs)
            wcols = wcp.tile([128, 32], F32R, tag="wcols")
            nc.gpsimd.tensor_copy(out=wcols, in_=zerof)
            nc.vector.tensor_copy(
                out=wcols[0:64, 0:10],
                in_=wtt[0:64, c, s8 * 10:(s8 + 1) * 10],
            )
            nc.vector.tensor_copy(
                out=wcols[64:128, 10:20],
                in_=wtt[64:128, c, s8 * 10:(s8 + 1) * 10],
            )
            nc.tensor.matmul(
                pc[0:32, 0:256], _r(wcols), _r(xr[:, c, :]),
                start=True, stop=True,
            )
            # stage this chunk's pooled outputs in T-layout
            xps = pps.tile([32, 512], F32, tag="xps")
            nc.scalar.copy(out=xps, in_=pc)
            for jc in range(2):
                ptp = ps_tp.tile([128, 128], F32, tag="tp", name="ptp")
                nc.tensor.transpose(
                    ptp[:, 0:32], xps[:, jc * 128:(jc + 1) * 128],
                    ident[0:32, 0:32],
                )
                nc.vector.tensor_copy(
                    out=staged_x[:, jc,
                                 s * 80 + c * 20:s * 80 + c * 20 + 20],
                    in_=ptp[:, 0:20],
                )
                ptq = ps_tp.tile([128, 128], F32, tag="tp", name="ptq")
                nc.tensor.transpose(
                    ptq[:, 0:32],
                    xps[:, 256 + jc * 128:256 + (jc + 1) * 128],
                    ident[0:32, 0:32],
                )
                nc.vector.tensor_copy(
                    out=staged_sf[:, jc, s * 8 + c * 2:s * 8 + c * 2 + 2],
                    in_=ptq[:, 0:2],
                )


def _post_body(
    nc, tc, graphs_pc, gchunks, dt, staged_x, staged_sf,
    ones1, ident, posw, pos, posT, ps_po, ps_pz, ps_pt, out_d,
):
    # post-stage weights (loaded after the main loop frees SBUF)
    cw = posw.tile([128, 16, 256], F32R, tag="cw")
    nc.sync.dma_start(out=cw, in_=dt["cw"].ap().rearrange("k p c -> p k c"))
    hwt = posw.tile([128, 4, 256], F32R, tag="hwt")
    nc.sync.dma_start(out=hwt, in_=dt["hw"].ap().rearrange("k p c -> p k c"))
    p1w = posw.tile([128, 6, 512], F32R, tag="p1w")
    nc.sync.dma_start(out=p1w, in_=dt["p1w"].ap().rearrange("k p c -> p k c"))
    p2w = posw.tile([128, 4, 256], F32R, tag="p2w")
    nc.sync.dma_start(out=p2w, in_=dt["p2w"].ap().rearrange("k p c -> p k c"))
    cbR = posw.tile([1, 256], F32R, tag="cbR")
    nc.sync.dma_start(out=cbR, in_=dt["cb"].ap())
    hbR = posw.tile([1, 256], F32R, tag="hbR")
    nc.sync.dma_start(out=hbR, in_=dt["hb"].ap())
    p1bR = posw.tile([1, 512], F32R, tag="p1bR")
    nc.sync.dma_start(out=p1bR, in_=dt["p1b"].ap())
    p2bR = posw.tile([1, 256], F32R, tag="p2bR")
    nc.sync.dma_start(out=p2bR, in_=dt["p2b"].ap())
    cgB = posw.tile([128, 256], F32, tag="cgB")
    _bcast(nc, cgB, dt["cg"])
    cbtB = posw.tile([128, 256], F32, tag="cbtB")
    _bcast(nc, cbtB, dt["cbt"])
    hgB = posw.tile([128, 256], F32, tag="hgB")
    _bcast(nc, hgB, dt["hg"])
    hbtB = posw.tile([128, 256], F32, tag="hbtB")
    _bcast(nc, hbtB, dt["hbt"])
    p1gB = posw.tile([128, 512], F32, tag="p1gB")
    _bcast(nc, p1gB, dt["p1g"])
    p1btB = posw.tile([128, 512], F32, tag="p1btB")
    _bcast(nc, p1btB, dt["p1bt"])

    sx3 = staged_x.rearrange("p k (g t) -> p k g t", t=10)

    catT = [posT.tile([128, graphs_pc], F32R, tag=f"catT{i}", name=f"catT{i}") for i in range(4)]
    zT = [posT.tile([128, graphs_pc], F32R, tag=f"zT{i}", name=f"zT{i}") for i in range(4)]
    pmv = posT.tile([128, 2 * gchunks, 2], F32, tag="pmv")

    # comb + hier matmuls, LN stats
    cps = []
    for gc in range(gchunks):
        gsl = slice(gc * 128, (gc + 1) * 128)
        cpp = ps_po.tile([128, 256], F32, tag="cpp")
        for h in range(H):
            for k in range(2):
                nc.tensor.matmul(
                    cpp, _r(sx3[:, k, gsl, h]), _r(cw[:, h * 2 + k, :]),
                    start=(h == 0 and k == 0), stop=False,
                )
        nc.tensor.matmul(cpp, _r(ones1), _r(cbR), start=False, stop=True)
        hpp = ps_po.tile([128, 256], F32, tag="cpp")
        for k in range(2):
            nc.tensor.matmul(
                hpp, _r(sx3[:, k, gsl, 8]), _r(hwt[:, k, :]),
                start=(k == 0), stop=False,
            )
            nc.tensor.matmul(
                hpp, _r(sx3[:, k, gsl, 9]), _r(hwt[:, 2 + k, :]),
                start=False, stop=(k == 1),
            )
        nc.tensor.matmul(hpp, _r(ones1), _r(hbR), start=False, stop=True)
        csb = posT.tile([128, 256], F32, tag=f"csb{gc}", name=f"csb{gc}")
        nc.scalar.copy(out=csb, in_=cpp)
        hsb = posT.tile([128, 256], F32, tag=f"hsb{gc}", name=f"hsb{gc}")
        nc.scalar.copy(out=hsb, in_=hpp)
        for i, ppx in enumerate((csb, hsb)):
            st6 = pos.tile([128, 6], F32, tag="pst6")
            nc.vector.bn_stats(out=st6, in_=ppx)
            nc.vector.bn_aggr(out=pmv[:, gc * 2 + i, :], in_=st6)
        cps.append((csb, hsb))

    prr = posT.tile([128, 2 * gchunks], F32, tag="prr")
    nc.vector.tensor_scalar(
        out=prr, in0=pmv[:, :, 1], scalar1=1.0, scalar2=1e-5,
        op0=OP.mult, op1=OP.add,
    )
    nc.scalar.activation(out=prr, in_=prr, func=AF.Sqrt)
    nc.vector.reciprocal(out=prr, in_=prr)

    cbundle = None  # populated below per gc
    for gc in range(gchunks):
        gsl = slice(gc * 128, (gc + 1) * 128)
        cpp, hpp = cps[gc]
        for i, (ppx, ggB, bbB) in enumerate(
            ((cpp, cgB, cbtB), (hpp, hgB, hbtB))
        ):
            tg = pos.tile([128, 256], F32, tag="ptg")
            nc.vector.scalar_tensor_tensor(
                out=tg, in0=ppx, scalar=pmv[:, gc * 2 + i, 0:1],
                in1=ggB, op0=OP.subtract, op1=OP.mult,
            )
            nc.vector.scalar_tensor_tensor(
                out=tg, in0=tg, scalar=prr[:, gc * 2 + i:gc * 2 + i + 1],
                in1=bbB, op0=OP.mult, op1=OP.add,
            )
            rg = pos.tile([128, 256], F32, tag="prg")
            nc.scalar.activation(out=rg, in_=tg, func=AF.Relu)
            for cc in range(2):
                ptp = ps_pt.tile([128, 128], F32, tag="pptp")
                nc.tensor.transpose(ptp, rg[:, cc * 128:(cc + 1) * 128], ident)
                nc.vector.tensor_copy(out=catT[i * 2 + cc][:, gsl], in_=ptp)

    catT_all = catT + [staged_sf[:, 0, :], staged_sf[:, 1, :]]

    # p1 matmul + LN + relu -> zT
    pmv2 = posT.tile([128, gchunks, 2], F32, tag="pmv2")
    zpps = []
    for gc in range(gchunks):
        gsl = slice(gc * 128, (gc + 1) * 128)
        zpp = ps_pz.tile([128, 512], F32, tag="zpp")
        for kk in range(6):
            nc.tensor.matmul(
                zpp, _r(catT_all[kk][:, gsl]), _r(p1w[:, kk, :]),
                start=(kk == 0), stop=False,
            )
        nc.tensor.matmul(zpp, _r(ones1), _r(p1bR), start=False, stop=True)
        zsb = posT.tile([128, 512], F32, tag=f"zsb{gc}", name=f"zsb{gc}")
        nc.scalar.copy(out=zsb, in_=zpp)
        st6 = pos.tile([128, 6], F32, tag="pst6")
        nc.vector.bn_stats(out=st6, in_=zsb)
        nc.vector.bn_aggr(out=pmv2[:, gc, :], in_=st6)
        zpps.append(zsb)

    prr2 = posT.tile([128, gchunks], F32, tag="prr2")
    nc.vector.tensor_scalar(
        out=prr2, in0=pmv2[:, :, 1], scalar1=1.0, scalar2=1e-5,
        op0=OP.mult, op1=OP.add,
    )
    nc.scalar.activation(out=prr2, in_=prr2, func=AF.Sqrt)
    nc.vector.reciprocal(out=prr2, in_=prr2)

    for gc in range(gchunks):
        gsl = slice(gc * 128, (gc + 1) * 128)
        zpp = zpps[gc]
        tg = pos.tile([128, 512], F32, tag="ptg5")
        nc.vector.scalar_tensor_tensor(
            out=tg, in0=zpp, scalar=pmv2[:, gc, 0:1],
            in1=p1gB, op0=OP.subtract, op1=OP.mult,
        )
        nc.vector.scalar_tensor_tensor(
            out=tg, in0=tg, scalar=prr2[:, gc:gc + 1],
            in1=p1btB, op0=OP.mult, op1=OP.add,
        )
        rg = pos.tile([128, 512], F32, tag="prg5")
        nc.scalar.activation(out=rg, in_=tg, func=AF.Relu)
        for kk in range(4):
            ptp = ps_pt.tile([128, 128], F32, tag="pptp")
            nc.tensor.transpose(ptp, rg[:, kk * 128:(kk + 1) * 128], ident)
            nc.vector.tensor_copy(out=zT[kk][:, gsl], in_=ptp)

    # final projection
    for gc in range(gchunks):
        gsl = slice(gc * 128, (gc + 1) * 128)
        opp = ps_po.tile([128, 256], F32, tag="cpp", name="opp")
        for kk in range(4):
            nc.tensor.matmul(
                opp, _r(zT[kk][:, gsl]), _r(p2w[:, kk, :]),
                start=(kk == 0), stop=False,
            )
        nc.tensor.matmul(opp, _r(ones1), _r(p2bR), start=False, stop=True)
        osb = pos.tile([128, 256], F32, tag="osb")
        nc.vector.tensor_copy(out=osb, in_=opp)
        nc.sync.dma_start(out=out_d.ap()[gsl, :], in_=osb)


# ---------------------------------------------------------------------------
# host side
# ---------------------------------------------------------------------------

_NC_CACHE = {}


def _get_nc(n_mega=FULL_N_MEGA):
    if n_mega not in _NC_CACHE:
        _NC_CACHE[n_mega] = build_nc(n_mega)
    return _NC_CACHE[n_mega]


def _prep_weights(inp):
    f = np.float32
    att_W1 = np.asarray(inp["att_W1"], f)          # [8, 256, 64]
    att_b1 = np.asarray(inp["att_b1"], f)          # [8, 64]
    att_w2 = np.asarray(inp["att_w2"], f)          # [8, 64]
    piece_W1 = np.asarray(inp["piece_W1"], f)      # [256, 128]
    empty_W1 = np.asarray(inp["empty_W1"], f)
    piece_b1 = np.asarray(inp["piece_b1"], f)      # [128]
    empty_b1 = np.asarray(inp["empty_b1"], f)
    piece_w2 = np.asarray(inp["piece_w2"], f)      # [128]
    empty_w2 = np.asarray(inp["empty_w2"], f)

    w1 = np.ascontiguousarray(
        np.transpose(att_W1, (1, 0, 2)).reshape(256, 512).reshape(2, 128, 512)
    )
    b1a = np.ascontiguousarray(att_b1.reshape(512).reshape(4, 128).T)
    pew = np.ascontiguousarray(
        np.concatenate([piece_W1, empty_W1], 1).reshape(2, 128, 256)
    )
    b1p = np.ascontiguousarray(
        np.concatenate([piece_b1, empty_b1]).reshape(2, 128).T
    )
    w2c = np.zeros((128, 4, 32), f)
    for h in range(H):
        m, half = divmod(h, 2)
        w2c[64 * half:64 * (half + 1), m, h] = att_w2[h]
    pw2c = np.zeros((128, 2, 32), f)
    pw2c[:, 0, 8] = piece_w2
    pw2c[:, 1, 9] = empty_w2
    b2 = np.zeros((1, 32), f)
    b2[0, 8] = np.float32(inp["piece_b2"])
    b2[0, 9] = np.float32(inp["empty_b2"])
    c = np.ascontiguousarray
    return {
        "w1": w1, "b1a": b1a, "pew": pew, "b1p": b1p,
        "w2c": w2c, "pw2c": pw2c, "b2": b2,
        "spw": c(np.asarray(inp["sp_W"], f).reshape(2, 128, 256)),
        "spb": c(np.asarray(inp["sp_b"], f).reshape(1, 256)),
        "spg": c(np.asarray(inp["sp_g"], f).reshape(1, 256)),
        "spbt": c(np.asarray(inp["sp_beta"], f).reshape(1, 256)),
        "sw": c((1.0 / (1.0 + np.exp(-np.asarray(inp["strat_w"], np.float64))))
                .astype(f).reshape(64, 1)),
        "cw": c(np.asarray(inp["comb_W"], f).reshape(16, 128, 256)),
        "cb": c(np.asarray(inp["comb_b"], f).reshape(1, 256)),
        "cg": c(np.asarray(inp["comb_g"], f).reshape(1, 256)),
        "cbt": c(np.asarray(inp["comb_beta"], f).reshape(1, 256)),
        "hw": c(np.asarray(inp["hier_W"], f).reshape(4, 128, 256)),
        "hb": c(np.asarray(inp["hier_b"], f).reshape(1, 256)),
        "hg": c(np.asarray(inp["hier_g"], f).reshape(1, 256)),
        "hbt": c(np.asarray(inp["hier_beta"], f).reshape(1, 256)),
        "p1w": c(np.asarray(inp["p1_W"], f).reshape(6, 128, 512)),
        "p1b": c(np.asarray(inp["p1_b"], f).reshape(1, 512)),
        "p1g": c(np.asarray(inp["p1_g"], f).reshape(1, 512)),
        "p1bt": c(np.asarray(inp["p1_beta"], f).reshape(1, 512)),
        "p2w": c(np.asarray(inp["p2_W"], f).reshape(4, 128, 256)),
        "p2b": c(np.asarray(inp["p2_b"], f).reshape(1, 256)),
    }


def make_in_maps(inputs, n_mega=FULL_N_MEGA):
    x = np.asarray(inputs["x"], np.float32)
    nt = np.asarray(inputs["node_types"]).astype(np.int32)
    wd = _prep_weights(inputs)
    nodes_pc = n_mega * MEGA * ST
    in_maps = []
    for c in range(N_CORES):
        m = {"x": np.ascontiguousarray(x[c * nodes_pc:(c + 1) * nodes_pc]),
             "nt": np.ascontiguousarray(nt[c * nodes_pc:(c + 1) * nodes_pc])}
        m.update(wd)
        in_maps.append(m)
    return in_maps


def run(inputs, n_mega=FULL_N_MEGA):
    nc = _get_nc(n_mega)
    in_maps = make_in_maps(inputs, n_mega)
    res = run_bass_kernel_spmd(nc, in_maps, core_ids=list(range(N_CORES)))
    return np.concatenate(
        [res.results[c]["out"] for c in range(N_CORES)], axis=0
    )


def kernel(**inputs):
    return run(inputs, FULL_N_MEGA)



# revision 1
# speedup vs baseline: 149.8834x; 149.8834x over previous
"""Trainium2 Bass kernel for nn_ChessGraphPooling (segment_reduce).

Data-parallel over whole graphs: 4096 boards x 64 nodes sharded across 8
NeuronCores (512 graphs / 32768 nodes per core); small weights replicated.
Per core, nodes are processed in "supertiles" of 512 nodes (4 partition
chunks of 128; 8 graphs), grouped in "megatiles" of 8 supertiles so the
segment softmax and LN-rstd work batch across 80 score rows at once.

Layouts:
  - x is kept in node-partition layout [128 nodes, C] (pooling matmuls and
    the strategic branch) and transposed on the PE into T-layout [C, nodes]
    tiles that feed the per-node linears as the moving operand.
  - scorer hidden activations are computed directly in T-layout so the
    scorer second layers become tiny matmuls and the segment softmax is a
    grouped free-axis reduction.
  - per-graph pooled features are staged in T-layout [C, graph*pool] so the
    post-pooling MLP stack runs as plain K-chunked matmuls.

Matmuls run as float32r (fp32 bits, full-rate rows at N>=256 on TRN2).
"""

import os
import sys

sys.path.insert(0, "/opt/trn_rl_repo")

from contextlib import ExitStack

import numpy as np

import concourse.bass as bass
import concourse.bacc as bacc
import concourse.tile as tile
import concourse.mybir as mybir
from concourse.bass_utils import run_bass_kernel_spmd
from concourse.masks import make_identity

F32 = mybir.dt.float32
# matmul dtype: float32r (full-rate rows) by default; K_MM_FP32=1 for plain fp32
F32R = mybir.dt.float32 if os.environ.get("K_MM_FP32") else mybir.dt.float32r
BF16 = mybir.dt.bfloat16
I32 = mybir.dt.int32
AF = mybir.ActivationFunctionType
OP = mybir.AluOpType
AX = mybir.AxisListType

C = 256
H = 8
NODES = 64
NEG = 0.2
N_CORES = 8
ST = 512          # nodes per supertile
CHUNKS = 4        # 128-node chunks per supertile
MEGA = 8          # supertiles per megatile (80 score rows)
FULL_N_MEGA = 8   # megatiles per core at full size


def _r(ap):
    return ap.bitcast(F32R)


def build_nc(n_mega=FULL_N_MEGA):
    nodes_pc = n_mega * MEGA * ST
    graphs_pc = nodes_pc // NODES
    assert graphs_pc % 128 == 0, "post stage needs graphs_pc multiple of 128"

    nc = bacc.Bacc("TRN2", num_devices=N_CORES)

    dt = {}

    def din(name, shape, dtype=F32):
        dt[name] = nc.dram_tensor(name, shape, dtype, kind="ExternalInput")

    din("x", [nodes_pc, C])
    din("nt", [nodes_pc], I32)
    din("w1", [2, 128, 512], F32R)
    din("b1a", [128, 4])
    din("pew", [2, 128, 256], F32R)
    din("b1p", [128, 2])
    din("w2c", [128, 4, 32], F32R)
    din("pw2c", [128, 2, 32], F32R)
    din("b2", [1, 32], F32R)
    din("spw", [2, 128, 256], F32R)
    din("spb", [1, 256], F32R)
    din("spg", [1, 256])
    din("spbt", [1, 256])
    din("sw", [64, 1])
    din("cw", [16, 128, 256], F32R)
    din("cb", [1, 256], F32R)
    din("cg", [1, 256])
    din("cbt", [1, 256])
    din("hw", [4, 128, 256], F32R)
    din("hb", [1, 256], F32R)
    din("hg", [1, 256])
    din("hbt", [1, 256])
    din("p1w", [6, 128, 512], F32R)
    din("p1b", [1, 512], F32R)
    din("p1g", [1, 512])
    din("p1bt", [1, 512])
    din("p2w", [4, 128, 256], F32R)
    din("p2b", [1, 256], F32R)
    out_d = nc.dram_tensor("out", [graphs_pc, C], F32, kind="ExternalOutput")

    with tile.TileContext(nc) as tc:
        _build_body(nc, tc, n_mega, graphs_pc, dt, out_d)
    nc.compile()
    return nc


def _bcast(nc, dst, src_d):
    """DMA a [1, D] dram row broadcast across dst's partitions."""
    nc.gpsimd.dma_start(
        out=dst, in_=src_d.ap().partition_broadcast(dst.shape[0])
    )


def _build_body(nc, tc, n_mega, graphs_pc, dt, out_d):
    gchunks = graphs_pc // 128

    with ExitStack() as top:
        consts = top.enter_context(tc.tile_pool(name="consts", bufs=1))
        persist = top.enter_context(tc.tile_pool(name="persist", bufs=1))

        # ---- constants for the main loop ----
        w1 = [consts.tile([128, 512], F32R, tag=f"w1_{k}", name=f"w1_{k}") for k in range(2)]
        pew = [consts.tile([128, 256], F32R, tag=f"pew_{k}", name=f"pew_{k}") for k in range(2)]
        spw = [consts.tile([128, 256], F32R, tag=f"spw_{k}", name=f"spw_{k}") for k in range(2)]
        for k in range(2):
            nc.sync.dma_start(out=w1[k], in_=dt["w1"].ap()[k])
            nc.sync.dma_start(out=pew[k], in_=dt["pew"].ap()[k])
            nc.sync.dma_start(out=spw[k], in_=dt["spw"].ap()[k])
        b1a = consts.tile([128, 4], F32)
        nc.sync.dma_start(out=b1a, in_=dt["b1a"].ap())
        b1p = consts.tile([128, 2], F32)
        nc.sync.dma_start(out=b1p, in_=dt["b1p"].ap())
        w2c = consts.tile([128, 4, 32], F32R)
        nc.sync.dma_start(out=w2c, in_=dt["w2c"].ap())
        pw2c = consts.tile([128, 2, 32], F32R)
        nc.sync.dma_start(out=pw2c, in_=dt["pw2c"].ap())
        b2 = consts.tile([1, 32], F32R)
        nc.sync.dma_start(out=b2, in_=dt["b2"].ap())
        spbR = consts.tile([1, 256], F32R)
        nc.sync.dma_start(out=spbR, in_=dt["spb"].ap())
        gB = consts.tile([128, 256], F32, tag="gB")
        _bcast(nc, gB, dt["spg"])
        btB = consts.tile([128, 256], F32, tag="btB")
        _bcast(nc, btB, dt["spbt"])

        sa = consts.tile([128, 1], F32, tag="sa")
        nc.sync.dma_start(out=sa[0:64, :], in_=dt["sw"].ap())
        nc.sync.dma_start(out=sa[64:128, :], in_=dt["sw"].ap())
        sa2 = consts.tile([128, 1], F32, tag="sa2")
        nc.vector.tensor_tensor(out=sa2, in0=sa, in1=sa, op=OP.mult)

        onesf = consts.tile([1, 512], F32, tag="onesf")
        nc.vector.memset(onesf, 1.0)
        ones512 = consts.tile([1, 512], F32R, tag="ones512")
        nc.vector.tensor_copy(out=ones512, in_=onesf)
        ones1 = consts.tile([1, 128], F32R, tag="ones1")
        nc.vector.tensor_copy(out=ones1, in_=onesf[:, 0:128])
        poolf = consts.tile([128, 32], F32, tag="poolf")
        nc.gpsimd.memset(poolf, 0.0)
        nc.gpsimd.memset(poolf[0:64, 0:1], 1.0 / NODES)
        nc.gpsimd.memset(poolf[64:128, 1:2], 1.0 / NODES)
        poolones = consts.tile([128, 32], F32R, tag="poolones")
        nc.vector.tensor_copy(out=poolones, in_=poolf)
        zerof = consts.tile([128, 32], F32, tag="zerof")
        nc.gpsimd.memset(zerof, 0.0)
        ident = consts.tile([128, 128], F32, tag="ident")
        make_identity(nc, ident)
        maskS = consts.tile([128, 512], F32, tag="maskS")
        nc.vector.memset(maskS, 1.0)

        # ---- persistent staging (alive into the post stage) ----
        staged_x = persist.tile([128, 2, graphs_pc * 10], F32R, tag="staged_x")
        staged_sf = persist.tile([128, 2, graphs_pc], F32R, tag="staged_sf")

        # ---- main loop ----
        with ExitStack() as main:
            xpool = main.enter_context(tc.tile_pool(name="xpool", bufs=9))
            xrp = main.enter_context(tc.tile_pool(name="xrp", bufs=2))
            t1pool = main.enter_context(tc.tile_pool(name="t1pool", bufs=9))
            actp = main.enter_context(tc.tile_pool(name="actp", bufs=4))
            sfp = main.enter_context(tc.tile_pool(name="sfp", bufs=3))
            scrp = main.enter_context(tc.tile_pool(name="scrp", bufs=3))
            wcp = main.enter_context(tc.tile_pool(name="wcp", bufs=3))
            megap = main.enter_context(tc.tile_pool(name="megap", bufs=2))
            pps = main.enter_context(tc.tile_pool(name="pps", bufs=2))

            ps_tp = main.enter_context(
                tc.tile_pool(name="ps_tp", bufs=2, space="PSUM"))
            ps_mm = main.enter_context(
                tc.tile_pool(name="ps_mm", bufs=2, space="PSUM"))
            ps_zs = main.enter_context(
                tc.tile_pool(name="ps_zs", bufs=1, space="PSUM"))
            ps_sc = main.enter_context(
                tc.tile_pool(name="ps_sc", bufs=1, space="PSUM"))
            ps_xs = main.enter_context(
                tc.tile_pool(name="ps_xs", bufs=2, space="PSUM"))

            for mega in range(n_mega):
                _mega_body(
                    nc, tc, mega, dt, staged_x, staged_sf,
                    w1, pew, spw, b1a, b1p, w2c, pw2c, b2, spbR, gB, btB,
                    sa, sa2, ones512, ones1, poolones, zerof, ident, maskS,
                    xpool, xrp, t1pool, actp, sfp, scrp, wcp, megap, pps,
                    ps_tp, ps_mm, ps_zs, ps_sc, ps_xs,
                )

        # ---- post stage ----
        with ExitStack() as post:
            posw = post.enter_context(tc.tile_pool(name="posw", bufs=1))
            pos = post.enter_context(tc.tile_pool(name="pos", bufs=4))
            posT = post.enter_context(tc.tile_pool(name="posT", bufs=1))
            ps_po = post.enter_context(
                tc.tile_pool(name="ps_po", bufs=2, space="PSUM"))
            ps_pz = post.enter_context(
                tc.tile_pool(name="ps_pz", bufs=2, space="PSUM"))
            ps_pt = post.enter_context(
                tc.tile_pool(name="ps_pt", bufs=2, space="PSUM"))
            _post_body(
                nc, tc, graphs_pc, gchunks, dt, staged_x, staged_sf,
                ones1, ident, posw, pos, posT, ps_po, ps_pz, ps_pt, out_d,
            )


def _mega_body(
    nc, tc, mega, dt, staged_x, staged_sf,
    w1, pew, spw, b1a, b1p, w2c, pw2c, b2, spbR, gB, btB,
    sa, sa2, ones512, ones1, poolones, zerof, ident, maskS,
    xpool, xrp, t1pool, actp, sfp, scrp, wcp, megap, pps,
    ps_tp, ps_mm, ps_zs, ps_sc, ps_xs,
):
    # masks: node_types -> maskS rows 10*j+8 (m) / 10*j+9 (1-m)
    ntm = megap.tile([8, 512], I32, tag="ntm")
    nc.sync.dma_start(
        out=ntm,
        in_=dt["nt"].ap()[mega * MEGA * ST:(mega + 1) * MEGA * ST]
        .rearrange("(s n) -> s n", s=8),
    )
    m8 = megap.tile([8, 512], F32, tag="m8")
    nc.vector.tensor_copy(out=m8, in_=ntm)
    n8 = megap.tile([8, 512], F32, tag="n8")
    nc.vector.tensor_scalar(
        out=n8, in0=m8, scalar1=-1.0, scalar2=1.0, op0=OP.mult, op1=OP.add
    )
    nc.sync.dma_start(out=maskS[8:88:10, :], in_=m8)
    nc.sync.dma_start(out=maskS[9:89:10, :], in_=n8)

    scstack = megap.tile([80, 512], F32, tag="scstack")
    mvs = megap.tile([128, MEGA, 4, 2], F32, tag="mvs")
    xs = []
    t1s = []
    for s8 in range(MEGA):
        s = mega * MEGA + s8
        xsb = xpool.tile([128, 4, 256], F32, tag="xsb")
        nc.sync.dma_start(
            out=xsb,
            in_=dt["x"].ap()[s * ST:(s + 1) * ST, :]
            .rearrange("(c p) m -> p c m", p=128),
        )
        xs.append(xsb)

        # transpose x chunks into T-layout [C, nodes]
        xT = [actp.tile([128, 512], F32R, tag="xT0", name="xT0"),
              actp.tile([128, 512], F32R, tag="xT1", name="xT1")]
        for c in range(CHUNKS):
            for k in range(2):
                tp = ps_tp.tile([128, 128], F32, tag="tp")
                nc.tensor.transpose(
                    tp, xsb[:, c, k * 128:(k + 1) * 128], ident
                )
                if (c * 2 + k) % 2 == 0:
                    nc.vector.tensor_copy(
                        out=xT[k][:, c * 128:(c + 1) * 128], in_=tp
                    )
                else:
                    nc.scalar.copy(
                        out=xT[k][:, c * 128:(c + 1) * 128], in_=tp
                    )

        scp = ps_sc.tile([32, 512], F32, tag="scp")
        # attention scorer: hT [hd, nodes] in 4 M-chunks; scores via matmul
        for m in range(4):
            ph = ps_mm.tile([128, 512], F32, tag="ph")
            nc.tensor.matmul(
                ph, _r(w1[0][:, m * 128:(m + 1) * 128]), _r(xT[0]),
                start=True, stop=False,
            )
            nc.tensor.matmul(
                ph, _r(w1[1][:, m * 128:(m + 1) * 128]), _r(xT[1]),
                start=False, stop=True,
            )
            hL = actp.tile([128, 512], F32R, tag="hL")
            nc.scalar.activation(
                out=hL, in_=ph, func=AF.Prelu,
                bias=b1a[:, m:m + 1], scale=1.0, alpha=NEG,
            )
            nc.tensor.matmul(
                scp, _r(w2c[:, m, :]), _r(hL),
                start=(m == 0), stop=False,
            )

        # piece/empty scorer: peT [pe-dim, nodes] in 2 M-chunks
        for m in range(2):
            pp = ps_mm.tile([128, 512], F32, tag="ph")
            nc.tensor.matmul(
                pp, _r(pew[0][:, m * 128:(m + 1) * 128]), _r(xT[0]),
                start=True, stop=False,
            )
            nc.tensor.matmul(
                pp, _r(pew[1][:, m * 128:(m + 1) * 128]), _r(xT[1]),
                start=False, stop=True,
            )
            peL = actp.tile([128, 512], F32R, tag="hL")
            nc.scalar.activation(
                out=peL, in_=pp, func=AF.Prelu,
                bias=b1p[:, m:m + 1], scale=1.0, alpha=NEG,
            )
            nc.tensor.matmul(
                scp, _r(pw2c[:, m, :]), _r(peL),
                start=False, stop=False,
            )
        nc.tensor.matmul(scp, _r(b2), _r(ones512), start=False, stop=True)
        stmp = scrp.tile([10, 512], F32, tag="stmp")
        nc.vector.tensor_copy(out=stmp, in_=scp[0:10, :])
        nc.sync.dma_start(out=scstack[s8 * 10:(s8 + 1) * 10, :], in_=stmp)

        # strategic branch: zs = x @ spW + spb (node layout), LN stats + center
        t1 = t1pool.tile([128, 4, 256], F32, tag="t1")
        t1s.append(t1)
        for cp in range(2):
            pz = ps_zs.tile([128, 512], F32, tag="pz")
            for half in range(2):
                c = cp * 2 + half
                sl = pz[:, half * 256:(half + 1) * 256]
                nc.tensor.matmul(
                    sl, _r(xT[0][:, c * 128:(c + 1) * 128]), _r(spw[0]),
                    start=True, stop=False,
                )
                nc.tensor.matmul(
                    sl, _r(xT[1][:, c * 128:(c + 1) * 128]), _r(spw[1]),
                    start=False, stop=False,
                )
                nc.tensor.matmul(
                    sl, _r(ones1), _r(spbR), start=False, stop=True,
                )
            for half in range(2):
                c = cp * 2 + half
                sl = pz[:, half * 256:(half + 1) * 256]
                st6 = scrp.tile([128, 6], F32, tag="st6")
                nc.vector.bn_stats(out=st6, in_=sl)
                nc.vector.bn_aggr(out=mvs[:, s8, c, :], in_=st6)
                # t1 = (zs - mu) * gamma  (sa-scaling folded into rho below)
                nc.vector.scalar_tensor_tensor(
                    out=t1[:, c], in0=sl, scalar=mvs[:, s8, c, 0:1],
                    in1=gB, op0=OP.subtract, op1=OP.mult,
                )

    # pe-score rows: (s + b2) * mask; att rows * 1.0
    nc.vector.tensor_tensor(
        out=scstack, in0=scstack, in1=maskS[0:80, :], op=OP.mult
    )

    # batched segment softmax over [80 rows, 8 graphs, 64 nodes]
    wT = megap.tile([80, 512], F32, tag="wT")
    mx = megap.tile([80, 8], F32, tag="mx")
    sc3 = scstack.rearrange("p (g n) -> p g n", n=NODES)
    wT3 = wT.rearrange("p (g n) -> p g n", n=NODES)
    nc.vector.tensor_reduce(out=mx, in_=sc3, axis=AX.X, op=OP.max)
    nc.vector.tensor_tensor(
        out=wT3, in0=sc3,
        in1=mx.unsqueeze(2).broadcast_to([80, 8, NODES]),
        op=OP.subtract,
    )
    nc.scalar.activation(out=wT, in_=wT, func=AF.Exp)
    dsum = megap.tile([80, 8], F32, tag="dsum")
    nc.vector.tensor_reduce(out=dsum, in_=wT3, axis=AX.X, op=OP.add)
    nc.vector.tensor_scalar(
        out=dsum, in0=dsum, scalar1=1e-16, scalar2=None, op0=OP.add,
    )
    nc.vector.reciprocal(out=dsum, in_=dsum)
    nc.vector.tensor_tensor(
        out=wT3, in0=wT3,
        in1=dsum.unsqueeze(2).broadcast_to([80, 8, NODES]),
        op=OP.mult,
    )

    # transpose the whole weight stack: wtt[:, c, r] = wT[r, c*128+:]
    wtt = megap.tile([128, 4, 80], F32, tag="wtt")
    for c in range(CHUNKS):
        tp = ps_tp.tile([128, 128], F32, tag="tp", name="wttp")
        nc.tensor.transpose(
            tp[:, 0:80], wT[:, c * 128:(c + 1) * 128], ident[0:80, 0:80]
        )
        nc.scalar.copy(out=wtt[:, c, :], in_=tp[:, 0:80])

    # batched LN scale: rho = sa / sqrt(sa^2 * var + eps)
    rho = megap.tile([128, 32], F32, tag="rho")
    nc.vector.tensor_scalar(
        out=rho, in0=mvs[:, :, :, 1].rearrange("p a b -> p (a b)"),
        scalar1=sa2, scalar2=1e-5, op0=OP.mult, op1=OP.add,
    )
    nc.scalar.activation(out=rho, in_=rho, func=AF.Sqrt)
    nc.vector.reciprocal(out=rho, in_=rho)
    nc.vector.tensor_scalar(
        out=rho, in0=rho, scalar1=sa, scalar2=None, op0=OP.mult
    )

    # phase C: finish strat LN, pool x and sf per supertile
    for s8 in range(MEGA):
        s = mega * MEGA + s8
        xsb = xs[s8]
        t1 = t1s[s8]
        xr = xrp.tile([128, 4, 256], F32R, tag="xr")
        nc.gpsimd.tensor_copy(out=xr, in_=xsb)
        for c in range(CHUNKS):
            # sf = relu(t1 * rho + beta)
            t2 = scrp.tile([128, 256], F32, tag="t2")
            nc.vector.scalar_tensor_tensor(
                out=t2, in0=t1[:, c],
                scalar=rho[:, s8 * 4 + c:s8 * 4 + c + 1],
                in1=btB, op0=OP.mult, op1=OP.add,
            )
            sf = sfp.tile([128, 256], F32R, tag="sf")
            nc.scalar.activation(out=sf, in_=t2, func=AF.Relu)
            pc = ps_xs.tile([32, 512], F32, tag="pc")
            nc.tensor.matmul(
                pc[0:32, 256:512], _r(poolones), _r(sf),
                start=True, stop=True,
            )
            # pooling weight columns for this chunk (block-diag over graphs)
            wcols = wcp.tile([128, 32], F32R, tag="wcols")
            nc.gpsimd.tensor_copy(out=wcols, in_=zerof)
            nc.vector.tensor_copy(
                out=wcols[0:64, 0:10],
                in_=wtt[0:64, c, s8 * 10:(s8 + 1) * 10],
            )
            nc.vector.tensor_copy(
                out=wcols[64:128, 10:20],
                in_=wtt[64:128, c, s8 * 10:(s8 + 1) * 10],
            )
            nc.tensor.matmul(
                pc[0:32, 0:256], _r(wcols), _r(xr[:, c, :]),
                start=True, stop=True,
            )
            # stage this chunk's pooled outputs in T-layout
            xps = pps.tile([32, 512], F32, tag="xps")
            nc.scalar.copy(out=xps, in_=pc)
            for jc in range(2):
                ptp = ps_tp.tile([128, 128], F32, tag="tp", name="ptp")
                nc.tensor.transpose(
                    ptp[:, 0:32], xps[:, jc * 128:(jc + 1) * 128],
                    ident[0:32, 0:32],
                )
                nc.vector.tensor_copy(
                    out=staged_x[:, jc,
                                 s * 80 + c * 20:s * 80 + c * 20 + 20],
                    in_=ptp[:, 0:20],
                )
                ptq = ps_tp.tile([128, 128], F32, tag="tp", name="ptq")
                nc.tensor.transpose(
                    ptq[:, 0:32],
                    xps[:, 256 + jc * 128:256 + (jc + 1) * 128],
                    ident[0:32, 0:32],
                )
                nc.vector.tensor_copy(
                    out=staged_sf[:, jc, s * 8 + c * 2:s * 8 + c * 2 + 2],
                    in_=ptq[:, 0:2],
                )


def _post_body(
    nc, tc, graphs_pc, gchunks, dt, staged_x, staged_sf,
    ones1, ident, posw, pos, posT, ps_po, ps_pz, ps_pt, out_d,
):
    # post-stage weights (loaded after the main loop frees SBUF)
    cw = posw.tile([128, 16, 256], F32R, tag="cw")
    nc.sync.dma_start(out=cw, in_=dt["cw"].ap().rearrange("k p c -> p k c"))
    hwt = posw.tile([128, 4, 256], F32R, tag="hwt")
    nc.sync.dma_start(out=hwt, in_=dt["hw"].ap().rearrange("k p c -> p k c"))
    p1w = posw.tile([128, 6, 512], F32R, tag="p1w")
    nc.sync.dma_start(out=p1w, in_=dt["p1w"].ap().rearrange("k p c -> p k c"))
    p2w = posw.tile([128, 4, 256], F32R, tag="p2w")
    nc.sync.dma_start(out=p2w, in_=dt["p2w"].ap().rearrange("k p c -> p k c"))
    cbR = posw.tile([1, 256], F32R, tag="cbR")
    nc.sync.dma_start(out=cbR, in_=dt["cb"].ap())
    hbR = posw.tile([1, 256], F32R, tag="hbR")
    nc.sync.dma_start(out=hbR, in_=dt["hb"].ap())
    p1bR = posw.tile([1, 512], F32R, tag="p1bR")
    nc.sync.dma_start(out=p1bR, in_=dt["p1b"].ap())
    p2bR = posw.tile([1, 256], F32R, tag="p2bR")
    nc.sync.dma_start(out=p2bR, in_=dt["p2b"].ap())
    cgB = posw.tile([128, 256], F32, tag="cgB")
    _bcast(nc, cgB, dt["cg"])
    cbtB = posw.tile([128, 256], F32, tag="cbtB")
    _bcast(nc, cbtB, dt["cbt"])
    hgB = posw.tile([128, 256], F32, tag="hgB")
    _bcast(nc, hgB, dt["hg"])
    hbtB = posw.tile([128, 256], F32, tag="hbtB")
    _bcast(nc, hbtB, dt["hbt"])
    p1gB = posw.tile([128, 512], F32, tag="p1gB")
    _bcast(nc, p1gB, dt["p1g"])
    p1btB = posw.tile([128, 512], F32, tag="p1btB")
    _bcast(nc, p1btB, dt["p1bt"])

    sx3 = staged_x.rearrange("p k (g t) -> p k g t", t=10)

    catT = [posT.tile([128, graphs_pc], F32R, tag=f"catT{i}", name=f"catT{i}") for i in range(4)]
    zT = [posT.tile([128, graphs_pc], F32R, tag=f"zT{i}", name=f"zT{i}") for i in range(4)]
    pmv = posT.tile([128, 2 * gchunks, 2], F32, tag="pmv")

    # comb + hier matmuls, LN stats
    cps = []
    for gc in range(gchunks):
        gsl = slice(gc * 128, (gc + 1) * 128)
        cpp = ps_po.tile([128, 256], F32, tag="cpp")
        for h in range(H):
            for k in range(2):
                nc.tensor.matmul(
                    cpp, _r(sx3[:, k, gsl, h]), _r(cw[:, h * 2 + k, :]),
                    start=(h == 0 and k == 0), stop=False,
                )
        nc.tensor.matmul(cpp, _r(ones1), _r(cbR), start=False, stop=True)
        hpp = ps_po.tile([128, 256], F32, tag="cpp")
        for k in range(2):
            nc.tensor.matmul(
                hpp, _r(sx3[:, k, gsl, 8]), _r(hwt[:, k, :]),
                start=(k == 0), stop=False,
            )
            nc.tensor.matmul(
                hpp, _r(sx3[:, k, gsl, 9]), _r(hwt[:, 2 + k, :]),
                start=False, stop=(k == 1),
            )
        nc.tensor.matmul(hpp, _r(ones1), _r(hbR), start=False, stop=True)
        csb = posT.tile([128, 256], F32, tag=f"csb{gc}", name=f"csb{gc}")
        nc.scalar.copy(out=csb, in_=cpp)
        hsb = posT.tile([128, 256], F32, tag=f"hsb{gc}", name=f"hsb{gc}")
        nc.scalar.copy(out=hsb, in_=hpp)
        for i, ppx in enumerate((csb, hsb)):
            st6 = pos.tile([128, 6], F32, tag="pst6")
            nc.vector.bn_stats(out=st6, in_=ppx)
            nc.vector.bn_aggr(out=pmv[:, gc * 2 + i, :], in_=st6)
        cps.append((csb, hsb))

    prr = posT.tile([128, 2 * gchunks], F32, tag="prr")
    nc.vector.tensor_scalar(
        out=prr, in0=pmv[:, :, 1], scalar1=1.0, scalar2=1e-5,
        op0=OP.mult, op1=OP.add,
    )
    nc.scalar.activation(out=prr, in_=prr, func=AF.Sqrt)
    nc.vector.reciprocal(out=prr, in_=prr)

    cbundle = None  # populated below per gc
    for gc in range(gchunks):
        gsl = slice(gc * 128, (gc + 1) * 128)
        cpp, hpp = cps[gc]
        for i, (ppx, ggB, bbB) in enumerate(
            ((cpp, cgB, cbtB), (hpp, hgB, hbtB))
        ):
            tg = pos.tile([128, 256], F32, tag="ptg")
            nc.vector.scalar_tensor_tensor(
                out=tg, in0=ppx, scalar=pmv[:, gc * 2 + i, 0:1],
                in1=ggB, op0=OP.subtract, op1=OP.mult,
            )
            nc.vector.scalar_tensor_tensor(
                out=tg, in0=tg, scalar=prr[:, gc * 2 + i:gc * 2 + i + 1],
                in1=bbB, op0=OP.mult, op1=OP.add,
            )
            rg = pos.tile([128, 256], F32, tag="prg")
            nc.scalar.activation(out=rg, in_=tg, func=AF.Relu)
            for cc in range(2):
                ptp = ps_pt.tile([128, 128], F32, tag="pptp")
                nc.tensor.transpose(ptp, rg[:, cc * 128:(cc + 1) * 128], ident)
                nc.vector.tensor_copy(out=catT[i * 2 + cc][:, gsl], in_=ptp)

    catT_all = catT + [staged_sf[:, 0, :], staged_sf[:, 1, :]]

    # p1 matmul + LN + relu -> zT
    pmv2 = posT.tile([128, gchunks, 2], F32, tag="pmv2")
    zpps = []
    for gc in range(gchunks):
        gsl = slice(gc * 128, (gc + 1) * 128)
        zpp = ps_pz.tile([128, 512], F32, tag="zpp")
        for kk in range(6):
            nc.tensor.matmul(
                zpp, _r(catT_all[kk][:, gsl]), _r(p1w[:, kk, :]),
                start=(kk == 0), stop=False,
            )
        nc.tensor.matmul(zpp, _r(ones1), _r(p1bR), start=False, stop=True)
        zsb = posT.tile([128, 512], F32, tag=f"zsb{gc}", name=f"zsb{gc}")
        nc.scalar.copy(out=zsb, in_=zpp)
        st6 = pos.tile([128, 6], F32, tag="pst6")
        nc.vector.bn_stats(out=st6, in_=zsb)
        nc.vector.bn_aggr(out=pmv2[:, gc, :], in_=st6)
        zpps.append(zsb)

    prr2 = posT.tile([128, gchunks], F32, tag="prr2")
    nc.vector.tensor_scalar(
        out=prr2, in0=pmv2[:, :, 1], scalar1=1.0, scalar2=1e-5,
        op0=OP.mult, op1=OP.add,
    )
    nc.scalar.activation(out=prr2, in_=prr2, func=AF.Sqrt)
    nc.vector.reciprocal(out=prr2, in_=prr2)

    for gc in range(gchunks):
        gsl = slice(gc * 128, (gc + 1) * 128)
        zpp = zpps[gc]
        tg = pos.tile([128, 512], F32, tag="ptg5")
        nc.vector.scalar_tensor_tensor(
            out=tg, in0=zpp, scalar=pmv2[:, gc, 0:1],
            in1=p1gB, op0=OP.subtract, op1=OP.mult,
        )
        nc.vector.scalar_tensor_tensor(
            out=tg, in0=tg, scalar=prr2[:, gc:gc + 1],
            in1=p1btB, op0=OP.mult, op1=OP.add,
        )
        rg = pos.tile([128, 512], F32, tag="prg5")
        nc.scalar.activation(out=rg, in_=tg, func=AF.Relu)
        for kk in range(4):
            ptp = ps_pt.tile([128, 128], F32, tag="pptp")
            nc.tensor.transpose(ptp, rg[:, kk * 128:(kk + 1) * 128], ident)
            nc.vector.tensor_copy(out=zT[kk][:, gsl], in_=ptp)

    # final projection
    for gc in range(gchunks):
        gsl = slice(gc * 128, (gc + 1) * 128)
        opp = ps_po.tile([128, 256], F32, tag="cpp", name="opp")
        for kk in range(4):
            nc.tensor.matmul(
                opp, _r(zT[kk][:, gsl]), _r(p2w[:, kk, :]),
                start=(kk == 0), stop=False,
            )
        nc.tensor.matmul(opp, _r(ones1), _r(p2bR), start=False, stop=True)
        osb = pos.tile([128, 256], F32, tag="osb")
        nc.vector.tensor_copy(out=osb, in_=opp)
        nc.sync.dma_start(out=out_d.ap()[gsl, :], in_=osb)


# ---------------------------------------------------------------------------
# host side
# ---------------------------------------------------------------------------

_NC_CACHE = {}


def _get_nc(n_mega=FULL_N_MEGA):
    if n_mega not in _NC_CACHE:
        _NC_CACHE[n_mega] = build_nc(n_mega)
    return _NC_CACHE[n_mega]


def _prep_weights(inp):
    f = np.float32
    att_W1 = np.asarray(inp["att_W1"], f)          # [8, 256, 64]
    att_b1 = np.asarray(inp["att_b1"], f)          # [8, 64]
    att_w2 = np.asarray(inp["att_w2"], f)          # [8, 64]
    piece_W1 = np.asarray(inp["piece_W1"], f)      # [256, 128]
    empty_W1 = np.asarray(inp["empty_W1"], f)
    piece_b1 = np.asarray(inp["piece_b1"], f)      # [128]
    empty_b1 = np.asarray(inp["empty_b1"], f)
    piece_w2 = np.asarray(inp["piece_w2"], f)      # [128]
    empty_w2 = np.asarray(inp["empty_w2"], f)

    w1 = np.ascontiguousarray(
        np.transpose(att_W1, (1, 0, 2)).reshape(256, 512).reshape(2, 128, 512)
    )
    b1a = np.ascontiguousarray(att_b1.reshape(512).reshape(4, 128).T)
    pew = np.ascontiguousarray(
        np.concatenate([piece_W1, empty_W1], 1).reshape(2, 128, 256)
    )
    b1p = np.ascontiguousarray(
        np.concatenate([piece_b1, empty_b1]).reshape(2, 128).T
    )
    w2c = np.zeros((128, 4, 32), f)
    for h in range(H):
        m, half = divmod(h, 2)
        w2c[64 * half:64 * (half + 1), m, h] = att_w2[h]
    pw2c = np.zeros((128, 2, 32), f)
    pw2c[:, 0, 8] = piece_w2
    pw2c[:, 1, 9] = empty_w2
    b2 = np.zeros((1, 32), f)
    b2[0, 8] = np.float32(inp["piece_b2"])
    b2[0, 9] = np.float32(inp["empty_b2"])
    c = np.ascontiguousarray
    return {
        "w1": w1, "b1a": b1a, "pew": pew, "b1p": b1p,
        "w2c": w2c, "pw2c": pw2c, "b2": b2,
        "spw": c(np.asarray(inp["sp_W"], f).reshape(2, 128, 256)),
        "spb": c(np.asarray(inp["sp_b"], f).reshape(1, 256)),
        "spg": c(np.asarray(inp["sp_g"], f).reshape(1, 256)),
        "spbt": c(np.asarray(inp["sp_beta"], f).reshape(1, 256)),
        "sw": c((1.0 / (1.0 + np.exp(-np.asarray(inp["strat_w"], np.float64))))
                .astype(f).reshape(64, 1)),
        "cw": c(np.asarray(inp["comb_W"], f).reshape(16, 128, 256)),
        "cb": c(np.asarray(inp["comb_b"], f).reshape(1, 256)),
        "cg": c(np.asarray(inp["comb_g"], f).reshape(1, 256)),
        "cbt": c(np.asarray(inp["comb_beta"], f).reshape(1, 256)),
        "hw": c(np.asarray(inp["hier_W"], f).reshape(4, 128, 256)),
        "hb": c(np.asarray(inp["hier_b"], f).reshape(1, 256)),
        "hg": c(np.asarray(inp["hier_g"], f).reshape(1, 256)),
        "hbt": c(np.asarray(inp["hier_beta"], f).reshape(1, 256)),
        "p1w": c(np.asarray(inp["p1_W"], f).reshape(6, 128, 512)),
        "p1b": c(np.asarray(inp["p1_b"], f).reshape(1, 512)),
        "p1g": c(np.asarray(inp["p1_g"], f).reshape(1, 512)),
        "p1bt": c(np.asarray(inp["p1_beta"], f).reshape(1, 512)),
        "p2w": c(np.asarray(inp["p2_W"], f).reshape(4, 128, 256)),
        "p2b": c(np.asarray(inp["p2_b"], f).reshape(1, 256)),
    }


def make_in_maps(inputs, n_mega=FULL_N_MEGA):
    x = np.asarray(inputs["x"], np.float32)
    nt = np.asarray(inputs["node_types"]).astype(np.int32)
    wd = _prep_weights(inputs)
    nodes_pc = n_mega * MEGA * ST
    in_maps = []
    for c in range(N_CORES):
        m = {"x": np.ascontiguousarray(x[c * nodes_pc:(c + 1) * nodes_pc]),
             "nt": np.ascontiguousarray(nt[c * nodes_pc:(c + 1) * nodes_pc])}
        m.update(wd)
        in_maps.append(m)
    return in_maps


def run(inputs, n_mega=FULL_N_MEGA):
    nc = _get_nc(n_mega)
    in_maps = make_in_maps(inputs, n_mega)
    res = run_bass_kernel_spmd(nc, in_maps, core_ids=list(range(N_CORES)))
    return np.concatenate(
        [res.results[c]["out"] for c in range(N_CORES)], axis=0
    )


def kernel(**inputs):
    return run(inputs, FULL_N_MEGA)

